# revision 8
# baseline (speedup 1.0000x reference)
"""Trainium2 Bass kernel for nn_AttentionModel (B=8, S=2048, D=1024).

Strategy: data-parallel over batch — core b computes batch b entirely
locally (no collectives).

All matmuls except the V projection run in fp8-e4m3 with DoubleRow
(256-deep contraction per MM, 2x bf16 TensorE throughput). The V
projection stays bf16 because the output is dominated by the +V residual;
fp8 there measures ~3.7% end-to-end error vs the 2% budget.

The K projection is folded away algebraically: scores = Q K^T =
X1 (Wq^T Wk) X2^T + u[q] + w2[k] + bq.bk. M = Wq^T Wk is precomputed on
the host (batch-independent weight folding), the device computes
T = X1 M and then T X2^T with raw fp8 X2 as the stationary operand. The
per-query term u[q] and the constant cancel in softmax; the per-key term
w2[k] folds into the per-partition exp bias next to the mask. M is
pre-scaled by 2^12 on the host so its small values stay in e4m3's normal
range; the 2^-12 folds into the PSUM-readout scale.

HW model (measured): the PE streams 1 moving-dim row per cycle at 2.4GHz
regardless of dtype (fp8 DR doubles contraction per row = 2x flops);
LDWEIGHTS is fully hidden. Total rows ~459k -> ~191us floor. The
remaining time is startup DMA latency, the p-state clock ramp, a ~4%
periodic activity throttle, and the fixed NEFF teardown. Three
countermeasures here:
  1. Host-blocked DRAM layouts ([g][p][r][cols], one 2-4KB contiguous
     run per partition per DR pair) so every input DMA is 128
     descriptors instead of 256-1024 fragmented ones; the first X1/W
     pairs land ~1.5us earlier.
  2. A warmup chain of dummy bf16 matmuls on a zeroed scratch tile runs
     while the first DMAs are in flight, walking the PE through its
     0.65/1.2GHz p-states so real matmuls start at full clock.
  3. The last attention q-tile splits its 1025-wide accumulation into
     two PSUM groups: the denominator + upper chunks finish first and
     their reciprocal/epilogue overlaps the final 8 matmuls, shortening
     the exposed tail to one STT + one full-width store.

DMA descriptor generation costs ~700ns per dma_start, serialized per
engine queue — startup transfers are spread critical-first across the
Sync (weights), Scalar (x1 g0/g2) and GpSimd (x1 g1/g3) queues.
"""

import numpy as np

B, S, D = 8, 2048, 1024
P = 128
NQ = 512                 # psum-bank-width matmul moving dim
SH = S // 2              # 1024, x3 half width
DT_TILES = D // P        # 8 dout tiles
KT_TILES = S // P        # 16 key tiles
GD = D // (2 * P)        # 4 contraction pairs over d
GK = S // (2 * P)        # 8 contraction pairs over keys
QW = 1024                # scores q-chunk width (2 matmuls per weight load)
N_QC = S // QW           # 2
V8W = 1040               # v8 inner stride: 1024 d + ones col + pad to %16
WSCALE = 4096.0          # host pre-scale on Wq/Wk before fp8 cast
SCALE = 1.0 / float(np.sqrt(D))
NEG_MASK = -30000.0
N_WARMUP = 14            # dummy bf16 MMs to ramp the PE p-state


def _apply_tile_patch():
    """This walrus build allows at most ONE semaphore wait on the tail
    CTRL/Drain instruction; Tile's kernel-tail drain carries one wait per
    touched logical proc. Spread them over multiple drains."""
    import copy

    from concourse import tile as _tile
    from concourse.vector_clock import ScopedClock as _ScopedClock

    if getattr(_tile.TileContext, "_drain_patch_applied", False):
        return

    def _patched(self, tick_clock, wait_clock):
        nc = self.nc
        drain_inst = nc.sync.drain()
        wait_clock.add_sem_waits(
            drain_inst.ins, _ScopedClock({None: tick_clock.global_clock})
        )
        mi = drain_inst.ins
        si = mi.sync_info
        waits = list(si.on_wait) if (si is not None and si.on_wait) else []
        if len(waits) > 1:
            si.on_wait = waits[:1]
            mi.sync_info = si
            for i in range(1, len(waits)):
                extra = nc.sync.drain()
                esi = copy.copy(si)
                esi.on_wait = [waits[i]]
                esi.on_update = []
                extra.ins.sync_info = esi

        nc.all_engine_barrier()
        assert self.sems is not None
        popped = nc._tile_sem_poison_stack.pop()
        assert popped is self._sem_poison
        nc.clear_and_free_semaphores(list(self.sems.allocated().values()))
        nc.all_engine_barrier()

    _tile.TileContext._drain_and_barrier = _patched
    _tile.TileContext._drain_patch_applied = True


def _split_excess_waits(nc, max_waits=1):
    """This walrus build rejects instructions carrying more than one
    semaphore wait ("Too many sync wait commands"). Hoist extra waits onto
    same-engine NoOp carriers inserted right before the instruction."""
    from concourse import mybir

    n_split = 0
    for f in nc.m.functions:
        for blk in f.blocks:
            insts = list(blk.instructions)
            out = []
            changed = False
            for inst in insts:
                si = inst.sync_info
                waits = list(si.on_wait) if (si is not None and si.on_wait) else []
                if len(waits) > max_waits:
                    head, tail = waits[:-max_waits], waits[-max_waits:]
                    for i in range(0, len(head), max_waits):
                        carrier = mybir.InstNoOp(
                            name=nc.get_next_instruction_name(),
                            engine=inst.engine,
                            ins=[],
                            outs=[],
                            sync_info=mybir.SyncInfo(
                                on_wait=head[i : i + max_waits], on_update=[]
                            ),
                        )
                        out.append(carrier)
                    si.on_wait = tail
                    inst.sync_info = si
                    changed = True
                    n_split += 1
                out.append(inst)
            if changed:
                blk.instructions = out
    return n_split


def _install_neff_cache():
    """Cache the NEFF keyed on the BIR json hash so repeat runs (same
    graph) skip the neuronx-cc compile."""
    import hashlib
    import os
    import shutil

    from concourse import bass2jax, bass_utils

    if getattr(bass_utils, "_neff_cache_installed", False):
        return
    orig = bass_utils.compile_bir_kernel

    def cached(bir_json, tmpdir, neff_name="file.neff"):
        h = hashlib.sha256(bytes(bir_json)).hexdigest()[:32]
        cdir = os.path.expanduser("~/.bass-neff-cache")
        os.makedirs(cdir, exist_ok=True)
        cpath = os.path.join(cdir, h + ".neff")
        if os.path.exists(cpath):
            dst = os.path.join(tmpdir, neff_name)
            shutil.copyfile(cpath, dst)
            return dst
        p = orig(bir_json, tmpdir, neff_name)
        try:
            shutil.copyfile(p, cpath)
        except OSError:
            pass
        return p

    bass_utils.compile_bir_kernel = cached
    bass2jax.compile_bir_kernel = cached
    bass_utils._neff_cache_installed = True


def build_nc(split_waits=True):
    """Build the per-core Bass graph (SPMD: same graph on all 8 cores)."""
    import concourse.bass as bass
    import concourse.tile as tile
    from concourse import mybir

    _apply_tile_patch()

    f32 = mybir.dt.float32
    bf16 = mybir.dt.bfloat16
    fp8 = mybir.dt.float8e4
    AF = mybir.ActivationFunctionType
    DR = mybir.MatmulPerfMode.DoubleRow
    ALU = mybir.AluOpType

    nc = bass.Bass()

    # Blocked operand layouts: row index = g*128 + p, inner = [r][cols] so
    # each DR pair's per-partition data is one contiguous 2-4KB run.
    x1b = nc.dram_tensor("x1b", [GD * P, 2 * S], fp8, kind="ExternalInput")
    x2b = nc.dram_tensor("x2b", [GD * P, 2 * S], fp8, kind="ExternalInput")
    x3b = nc.dram_tensor("x3b", [GD * P, 2 * S], bf16, kind="ExternalInput")
    # wqb holds the folded M = Wq^T @ Wk (scaled): scores = X1 M X2^T, so
    # no K projection runs on device. The per-query and constant bias
    # terms cancel in softmax; the per-key term is folded into cbias.
    wqb = nc.dram_tensor("wqb", [GD * P, 2 * D], fp8, kind="ExternalInput")
    wvb = nc.dram_tensor("wvb", [GD * P, 2 * D], bf16, kind="ExternalInput")
    # packed per-partition constant columns: mask+key-bias 0:16
    cbias = nc.dram_tensor("cbias", [P, KT_TILES], f32, kind="ExternalInput")
    bvr = nc.dram_tensor("bvr", [D], f32, kind="ExternalInput")
    out = nc.dram_tensor("out", [S, D], bf16, kind="ExternalOutput")

    def pair_dma(t_sb, blk2, src, g, width, src_col0=0, dst_col0=0, eng=None):
        """One DMA for DR pair g: SBUF t[:, blk2:blk2+2, dst_col0:+width]
        <- blocked dram rows [g*128, (g+1)*128) x [r 2][cols src_col0:+width).
        Full-width transfers are one contiguous 2*width run per
        partition (128 descriptors)."""
        src_ap = src[:]
        w = src_ap.ap[-1][1] // 2  # row length of the blocked tensor
        if width == w and src_col0 == 0:
            in_ap = bass.AP(
                tensor=src_ap.tensor,
                offset=src_ap.offset + g * P * 2 * w,
                ap=[[2 * w, P], [1, 2 * w]],
            )
        else:
            in_ap = bass.AP(
                tensor=src_ap.tensor,
                offset=src_ap.offset + g * P * 2 * w + src_col0,
                ap=[[2 * w, P], [w, 2], [1, width]],
            )
        (eng or nc.sync).dma_start(
            out=t_sb[:, blk2 : blk2 + 2, dst_col0 : dst_col0 + width],
            in_=in_ap,
        )

    with tile.TileContext(nc) as tc:
        with (
            tc.tile_pool(name="persist", bufs=1) as persist,
            tc.tile_pool(name="consts", bufs=1) as consts,
            tc.tile_pool(name="xin", bufs=1) as x_pool,
            tc.tile_pool(name="xv", bufs=2) as xv_pool,
            tc.tile_pool(name="wts", bufs=1) as w_pool,
            tc.tile_pool(name="es", bufs=2) as es_pool,
            tc.tile_pool(name="outp", bufs=4) as out_pool,
            tc.tile_pool(name="recp", bufs=4) as rec_pool,
            tc.tile_pool(name="psM", bufs=8, space="PSUM") as psM,
        ):
            # Persistent SBUF tensors (fused: middle axis = 128-row block).
            qt8 = persist.tile([P, DT_TILES, S], fp8, tag="qt8")
            kt8 = persist.tile([P, DT_TILES, S], fp8, tag="kt8")
            v_bf = persist.tile([P, KT_TILES, D], bf16, tag="vbf")
            v8 = persist.tile([P, KT_TILES, V8W], fp8, tag="v8")
            warm = persist.tile([P, NQ], bf16, tag="warm")

            cb_sb = consts.tile([P, KT_TILES], f32, tag="cb")
            bv_sb = consts.tile([P, D], f32, tag="bv")

            # --- PE p-state warmup: dummy matmuls on zeroed scratch while
            # the first operand DMAs are in flight. Results land in a
            # scratch PSUM bank; a 1-col reciprocal on DVE keeps them live
            # (emitted after the DMA issues so it doesn't block the
            # Vector queue's descriptor generation).
            wsink = consts.tile([P, 1], f32, tag="wsink")
            nc.vector.memset(warm[:, :], 0.0)
            psW = psM.tile([P, NQ], f32, tag="ps", name="ps_warm")
            for _ in range(N_WARMUP):
                nc.tensor.matmul(
                    psW[:], lhsT=warm[:, 0:P], rhs=warm[:, :],
                    start=True, stop=True,
                )

            # ones columns for the softmax denominator (all key blocks)
            nc.vector.memset(v8[:, :, D : D + 1], 1.0)

            mask_sb = cb_sb[:, 0:KT_TILES]

            # ---------------- Phase A: T projection ----------------
            # Startup DMAs, critical-first, spread over four queues (the
            # per-queue DMA stream moves ~512KB in ~3us, so each early
            # pair rides its own queue; pair g0 is split in halves).
            x1_sb = x_pool.tile([P, DT_TILES, S], fp8, tag="x1", name="x1")
            w_t = w_pool.tile([P, DT_TILES, D], fp8, tag="w8", name="w8")
            pair_dma(w_t, 0, wqb, 0, D)
            pair_dma(x1_sb, 0, x1b, 0, SH, eng=nc.scalar)
            pair_dma(x1_sb, 0, x1b, 0, SH, src_col0=SH, dst_col0=SH,
                     eng=nc.gpsimd)
            for g in range(1, GD):
                pair_dma(w_t, 2 * g, wqb, g, D)
            pair_dma(x1_sb, 2, x1b, 1, S, eng=nc.gpsimd)
            pair_dma(x1_sb, 4, x1b, 2, S, eng=nc.scalar)
            pair_dma(x1_sb, 6, x1b, 3, S, eng=nc.gpsimd)
            nc.scalar.dma_start(out=cb_sb[:], in_=cbias[:, :])
            nc.vector.reciprocal(out=wsink[:], in_=psW[:, 0:1])

            # T^T = M^T X1^T (fp8 DR), fused tiles [d, s]; kt8 needs no
            # compute at all — it's raw X2^T, DMA'd below.
            for jc in range(4):
                # g-major: 8 open accumulation groups (one per dout block)
                # so the first matmuls need only pair g0's data — the
                # startup ramp tracks the DMA feed pair-by-pair.
                ps8 = [
                    psM.tile([P, NQ], f32, tag="ps", name="ps_t")
                    for _ in range(DT_TILES)
                ]
                for g in range(GD):
                    rhs = x1_sb[:, 2 * g : 2 * g + 2, jc * NQ : (jc + 1) * NQ]
                    for di in range(DT_TILES):
                        nc.tensor.matmul(
                            ps8[di][:],
                            lhsT=w_t[:, 2 * g : 2 * g + 2, di * P : (di + 1) * P],
                            rhs=rhs,
                            start=(g == 0),
                            stop=(g == GD - 1),
                            perf_mode=DR,
                        )
                for di in range(DT_TILES):
                    nc.scalar.mul(
                        out=qt8[:, di, jc * NQ : (jc + 1) * NQ],
                        in_=ps8[di][:],
                        mul=1.0 / WSCALE,
                    )
                if jc == 1:
                    # prefetch the raw-X2 scores operand (needed in phase B)
                    for g in range(GD):
                        pair_dma(kt8, 2 * g, x2b, g, S)

            # --- V projection (bf16): fused out tiles [s, d] ---
            # Emitted between scores(qc0) and attnV(qc0): its 4MB of bf16
            # operands aren't needed until attnV, so deferring them off the
            # startup window relieves early DMA pressure.
            def emit_v_proj():
                bvr_ap = bvr[:]
                bv_bcast = bass.AP(
                    tensor=bvr_ap.tensor, offset=bvr_ap.offset,
                    ap=[[0, P], [1, D]],
                )
                nc.sync.dma_start(out=bv_sb[:], in_=bv_bcast)
                wv_t = w_pool.tile([P, DT_TILES, D], bf16, tag="wv", name="wv",
                                   bufs=1)
                for g in range(GD):
                    pair_dma(wv_t, 2 * g, wvb, g, D)
                for h in range(2):
                    xv_t = xv_pool.tile([P, DT_TILES, SH], bf16, tag="xv",
                                        name="xv", bufs=2)
                    for g in range(GD):
                        pair_dma(xv_t, 2 * g, x3b, g, SH, src_col0=h * SH)
                    for sl in range(KT_TILES // 2):
                        si = h * (KT_TILES // 2) + sl
                        ps2 = [
                            psM.tile([P, NQ], f32, tag="ps", name="ps_t")
                            for _ in range(2)
                        ]
                        for ii in range(DT_TILES):
                            lhsT = xv_t[:, ii, sl * P : (sl + 1) * P]
                            for dc in range(2):
                                nc.tensor.matmul(
                                    ps2[dc][:],
                                    lhsT=lhsT,
                                    rhs=wv_t[:, ii, dc * NQ : (dc + 1) * NQ],
                                    start=(ii == 0),
                                    stop=(ii == DT_TILES - 1),
                                )
                        for dc in range(2):
                            sl_d = slice(dc * NQ, (dc + 1) * NQ)
                            # psum + bv -> bf16 residual; ScalarE makes the
                            # fp8 matmul copy (DVE is the V-phase bottleneck)
                            nc.vector.tensor_add(
                                out=v_bf[:, si, sl_d], in0=ps2[dc][:],
                                in1=bv_sb[:, sl_d],
                            )
                            nc.scalar.activation(
                                out=v8[:, si, sl_d], in_=v_bf[:, si, sl_d],
                                func=AF.Copy,
                            )

            # NOTE: xv_t slicing above uses [:, ii, ...] with ii indexing
            # the fused middle axis; pairs were DMA'd to blocks 2g/2g+1 of
            # the same h-half, matching the baseline layout.

            # ---------------- Phase B: attention ----------------
            def emit_scores(qc):
                # scores^T fused tile for this q-chunk: [k 128, kb 16, q 1024]
                es_t = es_pool.tile([P, KT_TILES, QW], fp8, tag="es", name="es_t")
                for kb in range(KT_TILES):
                    ps2 = [
                        psM.tile([P, NQ], f32, tag="ps", name="ps_t")
                        for _ in range(2)
                    ]
                    for g in range(GD):
                        lhsT = kt8[:, 2 * g : 2 * g + 2, kb * P : (kb + 1) * P]
                        for j in range(2):
                            q0 = qc * QW + j * NQ
                            nc.tensor.matmul(
                                ps2[j][:],
                                lhsT=lhsT,
                                rhs=qt8[:, 2 * g : 2 * g + 2, q0 : q0 + NQ],
                                start=(g == 0),
                                stop=(g == GD - 1),
                                perf_mode=DR,
                            )
                    for j in range(2):
                        nc.scalar.activation(
                            out=es_t[:, kb, j * NQ : (j + 1) * NQ],
                            in_=ps2[j][:],
                            func=AF.Exp,
                            bias=mask_sb[:, kb : kb + 1],
                            scale=SCALE,
                        )
                return es_t

            def emit_attnv(qc, es_t):
                # attn @ V for the 8 query tiles of this chunk. The
                # [D + ones] = 1025-wide rhs is split into 3 ~342 chunks so
                # the denominator rides in the last chunk's final column —
                # N=1 matmuls cost a ~56ns pipeline bubble on the next MM.
                CH = ((0, 342), (342, 683), (683, 1025))
                for qi in range(QW // P):
                    qg = qc * (QW // P) + qi
                    last_q = (qc == N_QC - 1) and (qi == QW // P - 1)
                    o_sb = out_pool.tile([P, D], bf16, tag="ot", name="ot_t")
                    rec = rec_pool.tile([P, 1], f32, tag="rec", name="rec_t")
                    if not last_q:
                        po = [
                            psM.tile([P, c1 - c0], f32, tag="ps",
                                     name=f"psO_{dc}")
                            for dc, (c0, c1) in enumerate(CH)
                        ]
                        for g in range(GK):
                            lhsT = es_t[:, 2 * g : 2 * g + 2,
                                        qi * P : (qi + 1) * P]
                            st = (g == 0)
                            sp = (g == GK - 1)
                            # denominator chunk (dc=2) first so the
                            # reciprocal can start before the group's last
                            # matmul
                            for dc in (2, 0, 1):
                                c0, c1 = CH[dc]
                                nc.tensor.matmul(
                                    po[dc][:],
                                    lhsT=lhsT,
                                    rhs=v8[:, 2 * g : 2 * g + 2, c0:c1],
                                    start=st,
                                    stop=sp,
                                    perf_mode=DR,
                                )
                        nc.vector.reciprocal(out=rec[:], in_=po[2][:, 341:342])
                        for dc, (c0, c1) in enumerate(CH):
                            w = min(c1, D) - c0
                            sl_d = slice(c0, c0 + w)
                            # out = psum * (1/denom) + V in one fused op
                            nc.vector.scalar_tensor_tensor(
                                out=o_sb[:, sl_d],
                                in0=po[dc][:, 0:w],
                                scalar=rec[:],
                                in1=v_bf[:, qg, sl_d],
                                op0=ALU.mult,
                                op1=ALU.add,
                            )
                        nc.sync.dma_start(
                            out=out[qg * P : (qg + 1) * P, :], in_=o_sb[:],
                        )
                    else:
                        # Final tile: two accumulation groups. Group 1
                        # (upper chunks + denominator) finishes first; its
                        # reciprocal + epilogue overlap group 2's matmuls,
                        # so only one STT + the store trail the last MM.
                        po12 = [
                            psM.tile([P, c1 - c0], f32, tag="ps",
                                     name=f"psO_{dc}")
                            for dc, (c0, c1) in enumerate(CH[1:])
                        ]
                        for g in range(GK):
                            lhsT = es_t[:, 2 * g : 2 * g + 2,
                                        qi * P : (qi + 1) * P]
                            for i, (c0, c1) in ((1, CH[2]), (0, CH[1])):
                                nc.tensor.matmul(
                                    po12[i][:],
                                    lhsT=lhsT,
                                    rhs=v8[:, 2 * g : 2 * g + 2, c0:c1],
                                    start=(g == 0),
                                    stop=(g == GK - 1),
                                    perf_mode=DR,
                                )
                        nc.vector.reciprocal(out=rec[:], in_=po12[1][:, 341:342])
                        for i, (c0, c1) in ((0, CH[1]), (1, CH[2])):
                            w = min(c1, D) - c0
                            sl_d = slice(c0, c0 + w)
                            nc.vector.scalar_tensor_tensor(
                                out=o_sb[:, sl_d],
                                in0=po12[i][:, 0:w],
                                scalar=rec[:],
                                in1=v_bf[:, qg, sl_d],
                                op0=ALU.mult,
                                op1=ALU.add,
                            )
                        po0 = psM.tile([P, CH[0][1]], f32, tag="ps",
                                       name="psO_f")
                        for g in range(GK):
                            lhsT = es_t[:, 2 * g : 2 * g + 2,
                                        qi * P : (qi + 1) * P]
                            nc.tensor.matmul(
                                po0[:],
                                lhsT=lhsT,
                                rhs=v8[:, 2 * g : 2 * g + 2,
                                       CH[0][0] : CH[0][1]],
                                start=(g == 0),
                                stop=(g == GK - 1),
                                perf_mode=DR,
                            )
                        nc.vector.scalar_tensor_tensor(
                            out=o_sb[:, 0 : CH[0][1]],
                            in0=po0[:],
                            scalar=rec[:],
                            in1=v_bf[:, qg, 0 : CH[0][1]],
                            op0=ALU.mult,
                            op1=ALU.add,
                        )
                        nc.sync.dma_start(
                            out=out[qg * P : (qg + 1) * P, :], in_=o_sb[:],
                        )

            # Order: V-proj sits between scores(0) and attnV(0) so its 4MB
            # of operands load while scores compute, not during startup.
            es0 = emit_scores(0)
            emit_v_proj()
            emit_attnv(0, es0)
            es1 = emit_scores(1)
            emit_attnv(1, es1)

    if split_waits:
        _split_excess_waits(nc)
    return nc


def _prep_inputs(plms1, plms2, plms3, seqlengths, Wq, bq, Wk, bk, Wv, bv):
    """Host-side shard + layout prep. Returns in_maps for 8 cores."""
    import ml_dtypes

    bf = ml_dtypes.bfloat16
    f8 = ml_dtypes.float8_e4m3
    f32 = np.float32

    def blocked(a_T):  # [D, W] -> [D/2, 2W]: row g*128+p, inner [r][cols]
        Dd, W = a_T.shape
        return np.ascontiguousarray(
            a_T.reshape(GD, 2, P, W).transpose(0, 2, 1, 3).reshape(Dd // 2, 2 * W)
        )

    def t_(a):  # [S, D] -> [D, S] f32
        return np.ascontiguousarray(np.asarray(a, f32).T)

    Wq = np.asarray(Wq, f32)
    Wk = np.asarray(Wk, f32)
    bq = np.asarray(bq, f32)
    # Fold the K projection into the scores matmul: scores = Q K^T =
    # X1 (Wq^T Wk) X2^T + u[q] + w2[k] + bq.bk. The per-query term u and
    # the constant cancel in softmax; w2[k] = X2 (Wk^T bq) folds into the
    # per-key exp bias alongside the mask.
    M = Wq.T @ Wk
    wqb = blocked(np.clip(M * WSCALE, -240.0, 240.0)).astype(f8)
    v2 = Wk.T @ bq
    wvb = blocked(np.asarray(Wv, f32).T).astype(bf)
    bvr = np.asarray(bv, f32)
    seqlengths = np.asarray(seqlengths)

    in_maps = []
    ar = np.arange(S)
    for b in range(B):
        w2 = np.asarray(plms2)[b].astype(f32) @ v2
        keybias = np.where(
            ar < int(seqlengths[b]), SCALE * w2, NEG_MASK
        ).astype(f32)
        cbias = np.ascontiguousarray(keybias.reshape(KT_TILES, P).T)
        in_maps.append(
            {
                "x1b": blocked(t_(np.asarray(plms1)[b])).astype(f8),
                "x2b": blocked(t_(np.asarray(plms2)[b])).astype(f8),
                "x3b": blocked(t_(np.asarray(plms3)[b])).astype(bf),
                "wqb": wqb,
                "wvb": wvb,
                "cbias": cbias,
                "bvr": bvr,
            }
        )
    return in_maps


def kernel(**inputs) -> np.ndarray:
    from concourse.bass_utils import run_bass_kernel_spmd

    _install_neff_cache()

    in_maps = _prep_inputs(
        inputs["plms1"], inputs["plms2"], inputs["plms3"], inputs["seqlengths"],
        inputs["Wq"], inputs["bq"], inputs["Wk"], inputs["bk"],
        inputs["Wv"], inputs["bv"],
    )
    nc = build_nc()
    res = run_bass_kernel_spmd(nc, in_maps, core_ids=list(range(B)))
    return np.stack(
        [np.asarray(res.results[i]["out"], np.float32) for i in range(B)]
    )
